# revision 1
# baseline (speedup 1.0000x reference)
"""Trainium2 Bass kernel for nn_DelayLMLIFSNN (3-layer delay-conv + BN + LIF SNN).

Strategy:
- Data-parallel over batch B=64 across 8 NeuronCores (8 batch elements/core).
- Per layer: causal dilated-gauss conv as 100 PE matmuls per (h-half, batch)
  [fp16 weight-split x2: w = w0 + w1*2^-12, binary spike inputs are exact in
  fp16, fp32 PSUM accumulation => ~fp32-exact conv],
  BatchNorm stats via cross-core AllReduce (two-pass mean/var, Newton-refined
  1/sqrt), then the LIF soft-reset scan: 512 serial steps x 4 DVE ops on
  (128p x 16f) state tiles, matching the reference's fp32 rounding order.
"""

import os
import numpy as np

T, B, J, H, K, NL = 512, 64, 256, 256, 25, 3
THETA = 1.0
SIGMA_INIT = 0.5
EPS = 1e-5
BL = B // 8          # batch per core
NBLK = 2 * BL        # (b, c) blocks per core
TPAD = T + (K - 1)   # left-padded time for conv input

_CACHE = {}
LAST = {"exec_time_ns": None, "results": None}


# ----------------------------------------------------------------------------
# Host-side math
# ----------------------------------------------------------------------------

def _gauss_kernel_host(W, P):
    """Replicates reference.gauss_kernel in fp32. Uses jax-cpu when available
    so host arithmetic bit-matches the jax reference; numpy fallback."""
    try:
        import jax
        import jax.numpy as jnp

        cpu = jax.devices("cpu")[0]

        def gk(W, P):
            pos = jnp.arange(K, dtype=W.dtype)
            c = P + K // 2
            s = jnp.abs(jnp.float32(SIGMA_INIT)) + 0.27
            g = jnp.exp(-0.5 * ((pos[None, None, :] - c[..., None]) / s) ** 2)
            g = g / (jnp.sum(g, axis=-1, keepdims=True) + 1e-7)
            return W[..., None] * g

        with jax.default_device(cpu):
            return np.array(jax.jit(gk, backend="cpu")(jnp.asarray(W), jnp.asarray(P)))
    except Exception:
        pos = np.arange(K, dtype=np.float32)
        c = (P + np.float32(K // 2)).astype(np.float32)
        s = np.float32(abs(SIGMA_INIT) + 0.27)
        t = ((pos[None, None, :] - c[..., None]) / s).astype(np.float32)
        g = np.exp((np.float32(-0.5) * (t * t)).astype(np.float32)).astype(np.float32)
        den = (np.sum(g, axis=-1, keepdims=True, dtype=np.float32) + np.float32(1e-7)).astype(np.float32)
        g = (g / den).astype(np.float32)
        return (W[..., None] * g).astype(np.float32)


def _fp16_split(kern):
    """kern (fp32) -> (w0, w1) fp16 with kern ~= w0 + w1 * 2^-12 (residual
    <= ~2^-24*|kern|). Subnormal fp16 values are flushed host-side so PE
    flush-to-zero behavior (if any) cannot bite."""
    FP16_MIN_NORMAL = 6.104e-5
    w0 = kern.astype(np.float16)
    w0 = np.where(np.abs(w0.astype(np.float32)) < FP16_MIN_NORMAL, np.float16(0), w0)
    r = (kern - w0.astype(np.float32)) * np.float32(4096.0)
    w1 = r.astype(np.float16)
    w1 = np.where(np.abs(w1.astype(np.float32)) < FP16_MIN_NORMAL, np.float16(0), w1)
    return w0, w1


def _prep_static(W, P, beta, gamma, bb, U0):
    """Build the replicated parameter arrays (same on all cores)."""
    # weights: (l, c_outhalf, split, j_partition, jc, k, h) fp16
    wts = np.empty((NL, 2, 2, 128, 2, K, 128), np.float16)
    for l in range(NL):
        kern = _gauss_kernel_host(W[l], P[l])      # (H, J, K) fp32
        w0, w1 = _fp16_split(kern)
        for c in range(2):
            for s, wsrc in enumerate((w0, w1)):
                # lhsT[j, h] = w[c*128+h, jc*128+j, k]
                blk = wsrc[c * 128:(c + 1) * 128]          # (128h, J, K)
                # -> (j, jc, k, h)
                arr = blk.reshape(128, 2, 128, K)          # (h, jc, j, k)
                wts[l, c, s] = arr.transpose(2, 1, 3, 0)   # (j, jc, k, h)

    betat = np.empty((NL, 128, 16), np.float32)
    ombcol = np.empty((NL, 128, 2), np.float32)
    gcol = np.empty((NL, 128, 2), np.float32)
    bcol = np.empty((NL, 128, 2), np.float32)
    for l in range(NL):
        for c in range(2):
            ch = beta[l, c * 128:(c + 1) * 128].astype(np.float32)
            for b in range(BL):
                betat[l, :, b * 2 + c] = ch
            ombcol[l, :, c] = (np.float32(1.0) - ch).astype(np.float32)
            gcol[l, :, c] = gamma[l, c * 128:(c + 1) * 128]
            bcol[l, :, c] = bb[l, c * 128:(c + 1) * 128]
    return wts, betat, ombcol, gcol, bcol


def _prep_percore(x, U0, core):
    x16 = np.zeros((128, NBLK, TPAD), np.float16)
    xs = x[:, core * BL:(core + 1) * BL, :]            # (T, BL, J)
    # x16[p, b*2+jc, 24+t] = x[t, b, jc*128+p]
    a = xs.astype(np.float16).reshape(T, BL, 2, 128)   # (t, b, jc, p)
    x16[:, :, K - 1:] = a.transpose(3, 1, 2, 0).reshape(128, NBLK, T)

    d0 = np.empty((NL, 128, 16), np.float32)
    u = U0[:, core * BL:(core + 1) * BL, :]            # (NL, BL, H)
    for l in range(NL):
        a = u[l].reshape(BL, 2, 128)                   # (b, c, p)
        d0[l] = a.transpose(2, 0, 1).reshape(128, 16)
    return x16, d0


# ----------------------------------------------------------------------------
# Device program
# ----------------------------------------------------------------------------

def _build(nc, bn_affine_trivial):
    import concourse.tile as tile
    from concourse import mybir
    from contextlib import ExitStack

    F32 = mybir.dt.float32
    F16 = mybir.dt.float16
    AX = mybir.AxisListType
    OP = mybir.AluOpType
    SQRT = mybir.ActivationFunctionType.Sqrt

    ap_x = nc.dram_tensor("x16", [128, NBLK, TPAD], F16, kind="ExternalInput").ap()
    ap_w = nc.dram_tensor("wts", [NL, 2, 2, 128, 2, K, 128], F16, kind="ExternalInput").ap()
    ap_d0 = nc.dram_tensor("d0", [NL, 128, 16], F32, kind="ExternalInput").ap()
    ap_bt = nc.dram_tensor("betat", [NL, 128, 16], F32, kind="ExternalInput").ap()
    ap_omb = nc.dram_tensor("ombcol", [NL, 128, 2], F32, kind="ExternalInput").ap()
    ap_g = nc.dram_tensor("gcol", [NL, 128, 2], F32, kind="ExternalInput").ap()
    ap_bb = nc.dram_tensor("bcol", [NL, 128, 2], F32, kind="ExternalInput").ap()
    ap_out = nc.dram_tensor("out", [2, 128, BL, T], F32, kind="ExternalOutput").ap()

    with tile.TileContext(nc) as tc, ExitStack() as ctx:
        wp = ctx.enter_context(tc.tile_pool(name="wp", bufs=4))
        xp = ctx.enter_context(tc.tile_pool(name="xp", bufs=2))
        yp = ctx.enter_context(tc.tile_pool(name="yp", bufs=1))
        sp = ctx.enter_context(tc.tile_pool(name="sp", bufs=2))
        st = ctx.enter_context(tc.tile_pool(name="st", bufs=2))
        ps = ctx.enter_context(tc.tile_pool(name="ps", bufs=4, space="PSUM"))
        dr = ctx.enter_context(tc.tile_pool(name="dr", bufs=4, space="DRAM"))

        xcur = xp.tile([128, NBLK, TPAD], F16, tag="xreg")
        nc.sync.dma_start(xcur[:], ap_x)

        for l in range(NL):
            # ---------------- conv ----------------
            yreg = yp.tile([128, NBLK, T], F32, tag="yreg")
            for c in range(2):
                wt0 = wp.tile([128, 2, K, 128], F16, tag="w")
                wt1 = wp.tile([128, 2, K, 128], F16, tag="w")
                nc.sync.dma_start(wt0[:], ap_w[l, c, 0])
                nc.sync.dma_start(wt1[:], ap_w[l, c, 1])
                for b in range(BL):
                    p0 = ps.tile([128, T], F32, tag="psum")
                    p1 = ps.tile([128, T], F32, tag="psum")
                    for s, (pt, wt) in enumerate(((p0, wt0), (p1, wt1))):
                        for jc in range(2):
                            for k in range(K):
                                nc.tensor.matmul(
                                    pt[:],
                                    lhsT=wt[:, jc, k, :],
                                    rhs=xcur[:, b * 2 + jc, k:k + T],
                                    start=(jc == 0 and k == 0),
                                    stop=(jc == 1 and k == K - 1),
                                )
                    yblk = yreg[:, b * 2 + c, :]
                    nc.vector.tensor_scalar(yblk, p1[:], float(2.0 ** -12), None, OP.mult)
                    nc.vector.tensor_tensor(yblk, yblk, p0[:], OP.add)

            # ---------------- BN stats: mean ----------------
            sred = st.tile([128, 16], F32, tag="sred")
            for m in range(NBLK):
                nc.vector.tensor_reduce(sred[:, m:m + 1], yreg[:, m, :], AX.X, OP.add)
            ssum = st.tile([128, 2], F32, tag="ssum")
            for c in range(2):
                nc.vector.tensor_reduce(ssum[:, c:c + 1], sred[:, c::2], AX.X, OP.add)
            cin1 = dr.tile([128, 2], F32, tag="cc")
            cout1 = dr.tile([128, 2], F32, tag="cc")
            nc.sync.dma_start(cin1[:], ssum[:])
            nc.gpsimd.collective_compute(
                "AllReduce", OP.add, replica_groups=[list(range(8))],
                ins=[cin1[:].opt()], outs=[cout1[:].opt()])
            gsum = st.tile([128, 2], F32, tag="gsum")
            nc.sync.dma_start(gsum[:], cout1[:])
            mu = st.tile([128, 2], F32, tag="mu")
            nc.vector.tensor_scalar(mu[:], gsum[:], float(1.0 / (T * B)), None, OP.mult)

            # ---------------- center + variance ----------------
            SQUARE = mybir.ActivationFunctionType.Square
            vred = st.tile([128, 16], F32, tag="vred")
            for c in range(2):
                nc.vector.tensor_scalar(
                    yreg[:, c::2, :], yreg[:, c::2, :], mu[:, c:c + 1], None, OP.subtract)
            for m in range(NBLK):
                sq = sp.tile([128, T], F32, tag="sq")
                nc.scalar.activation(sq[:], yreg[:, m, :], SQUARE,
                                     accum_out=vred[:, m:m + 1])
            vsum = st.tile([128, 2], F32, tag="vsum")
            for c in range(2):
                nc.vector.tensor_reduce(vsum[:, c:c + 1], vred[:, c::2], AX.X, OP.add)
            cin2 = dr.tile([128, 2], F32, tag="cc")
            cout2 = dr.tile([128, 2], F32, tag="cc")
            nc.sync.dma_start(cin2[:], vsum[:])
            nc.gpsimd.collective_compute(
                "AllReduce", OP.add, replica_groups=[list(range(8))],
                ins=[cin2[:].opt()], outs=[cout2[:].opt()])
            gvs = st.tile([128, 2], F32, tag="gvs")
            nc.sync.dma_start(gvs[:], cout2[:])

            # v = var + eps ; s = sqrt(v) via ACT seed + 2 Newton iters;
            # r = 1/s via DVE HW divide
            v = st.tile([128, 2], F32, tag="v")
            nc.vector.tensor_scalar(v[:], gvs[:], float(1.0 / (T * B)), None, OP.mult)
            nc.vector.tensor_scalar(v[:], v[:], float(EPS), None, OP.add)
            sqt = st.tile([128, 2], F32, tag="sqt")
            rcp = st.tile([128, 2], F32, tag="rcp")
            qt = st.tile([128, 2], F32, tag="qt")
            nc.scalar.activation(sqt[:], v[:], SQRT)
            for _ in range(2):
                nc.vector.reciprocal(rcp[:], sqt[:])
                nc.vector.tensor_tensor(qt[:], v[:], rcp[:], OP.mult)
                nc.vector.tensor_tensor(sqt[:], sqt[:], qt[:], OP.add)
                nc.vector.tensor_scalar(sqt[:], sqt[:], 0.5, None, OP.mult)
            rr = st.tile([128, 2], F32, tag="rr")
            nc.vector.reciprocal(rr[:], sqt[:])

            # ---------------- z = ((d*r)*gamma + bb) * (1-beta) ----------------
            ombc = st.tile([128, 2], F32, tag="ombc")
            nc.sync.dma_start(ombc[:], ap_omb[l])
            if not bn_affine_trivial:
                gc = st.tile([128, 2], F32, tag="gc")
                bc = st.tile([128, 2], F32, tag="bc")
                nc.sync.dma_start(gc[:], ap_g[l])
                nc.sync.dma_start(bc[:], ap_bb[l])
            for c in range(2):
                blk = yreg[:, c::2, :]
                nc.vector.tensor_scalar(blk, blk, rr[:, c:c + 1], None, OP.mult)
                if not bn_affine_trivial:
                    nc.vector.tensor_scalar(blk, blk, gc[:, c:c + 1], None, OP.mult)
                    nc.vector.tensor_scalar(blk, blk, bc[:, c:c + 1], None, OP.add)
                nc.vector.tensor_scalar(blk, blk, ombc[:, c:c + 1], None, OP.mult)

            # ---------------- LIF scan ----------------
            Dt = st.tile([128, 16], F32, tag="D")
            Ut = st.tile([128, 16], F32, tag="U")
            bt = st.tile([128, 16], F32, tag="bt")
            nc.sync.dma_start(Dt[:], ap_d0[l])
            nc.sync.dma_start(bt[:], ap_bt[l])
            for t in range(T):
                zcol = yreg[:, :, t]
                nc.vector.tensor_tensor(Ut[:], Dt[:], bt[:], OP.mult)
                nc.vector.tensor_tensor(Ut[:], Ut[:], zcol, OP.add)
                nc.vector.tensor_scalar(zcol, Ut[:], float(THETA), None, OP.is_gt)
                nc.vector.tensor_tensor(Dt[:], Ut[:], zcol, OP.subtract)

            # ---------------- spikes out ----------------
            if l < NL - 1:
                xnext = xp.tile([128, NBLK, TPAD], F16, tag="xreg")
                nc.vector.memset(xnext[:, :, 0:K - 1], 0.0)
                nc.vector.tensor_copy(xnext[:, :, K - 1:], yreg[:])
                xcur = xnext
            else:
                for c in range(2):
                    nc.sync.dma_start(ap_out[c], yreg[:, c::2, :])
    nc.compile()
    return nc


def _get_compiled(bn_affine_trivial):
    key = ("prog", bn_affine_trivial)
    if key not in _CACHE:
        from concourse import bacc
        nc = bacc.Bacc("TRN2", target_bir_lowering=False, debug=False, num_devices=8)
        _CACHE[key] = _build(nc, bn_affine_trivial)
    return _CACHE[key]


# ----------------------------------------------------------------------------
# Profiled run (dev-only; needs the axon NTFF side channel)
# ----------------------------------------------------------------------------

def _run_profiled(nc, in_maps):
    import glob
    import tempfile
    from concourse.bass_utils import run_bass_kernel_spmd

    prof = {}
    try:
        from trn_agent_boot.trn_boot import _ntff_profile_via_ctypes
        hook = _ntff_profile_via_ctypes("/opt/axon/libaxon_pjrt.so")
        assert hook is not None
        neff_dir = tempfile.mkdtemp(prefix="snn_ntff_")
        with hook(neff_dir, [0]):
            res = run_bass_kernel_spmd(nc, in_maps, list(range(8)))
        ntffs = glob.glob(os.path.join(neff_dir, "*_body*.ntff"))
        prof["neff_dir"] = neff_dir
        if ntffs:
            import gauge.profiler
            from concourse._compat import FishPath
            p = gauge.profiler.Profile(
                profile_path=FishPath(neff_dir), kernel_dev_mode=True,
                profile_on_exit=False, bass_kernel=nc.m,
                offline_processing=True, fname="*_body*")
            rs = p.to_perfetto(model_index=(0,))
            if rs:
                prof["exec_time_ns"] = rs[0].exec_time_ns
                prof["trace_path"] = str(rs[0].trace_path)
                prof["scope_times"] = dict(rs[0].scope_times)
        return res, prof
    except Exception as e:  # profiling is best-effort
        prof["error"] = repr(e)
        res = run_bass_kernel_spmd(nc, in_maps, list(range(8)))
        return res, prof


# ----------------------------------------------------------------------------
# Entry point
# ----------------------------------------------------------------------------

def kernel(x, W, P, beta, gamma, bb, U0):
    from concourse.bass_utils import run_bass_kernel_spmd

    x = np.asarray(x, np.float32)
    W = np.asarray(W, np.float32)
    P = np.asarray(P, np.float32)
    beta = np.asarray(beta, np.float32)
    gamma = np.asarray(gamma, np.float32)
    bb = np.asarray(bb, np.float32)
    U0 = np.asarray(U0, np.float32)

    trivial = bool(np.all(gamma == 1.0) and np.all(bb == 0.0))
    nc = _get_compiled(trivial)

    skey = ("static", W.tobytes(), P.tobytes(), beta.tobytes(),
            gamma.tobytes(), bb.tobytes())
    sk = hash(skey)
    if _CACHE.get("static_key") != sk:
        _CACHE["static"] = _prep_static(W, P, beta, gamma, bb, U0)
        _CACHE["static_key"] = sk
    wts, betat, ombcol, gcol, bcol = _CACHE["static"]

    in_maps = []
    for core in range(8):
        x16, d0 = _prep_percore(x, U0, core)
        in_maps.append(dict(x16=x16, wts=wts, d0=d0, betat=betat,
                            ombcol=ombcol, gcol=gcol, bcol=bcol))

    trace = bool(int(os.environ.get("BASS_SNN_TRACE", "0")))
    if trace:
        res, prof = _run_profiled(nc, in_maps)
        LAST["exec_time_ns"] = prof.get("exec_time_ns")
        LAST["profile"] = prof
    else:
        res = run_bass_kernel_spmd(nc, in_maps, list(range(8)))
        LAST["exec_time_ns"] = res.exec_time_ns
    LAST["results"] = res

    o = np.empty((T, B, H), np.float32)
    for core in range(8):
        arr = res.results[core]["out"]                  # (2, 128, BL, T)
        o[:, core * BL:(core + 1) * BL, :] = (
            arr.transpose(3, 2, 0, 1).reshape(T, BL, H))
    return o



# revision 2
# speedup vs baseline: 1.5981x; 1.5981x over previous
"""Trainium2 Bass kernel for nn_DelayLMLIFSNN (3-layer delay-conv + BN + LIF SNN).

Strategy (v2):
- Data-parallel over batch B=64 across 8 NeuronCores (8 batch elements/core).
- Conv: causal dilated-gauss conv as PE matmuls, fp16 weight-split
  (w = w0 + w1*2^-12) accumulated in a SINGLE psum chain per output block by
  pre-scaling a second copy of the (binary, fp16-exact) input: xs = x*2^-12.
- BN stats fused into the conv epilogue on the Scalar engine (Copy+accum for
  the sum, Square+accum for the sumsq), one-pass var = E[y^2]-mu^2, a single
  [128,4] AllReduce per layer, plus a warmup AllReduce at kernel start that
  absorbs inter-core start skew.
- LIF scan: per-step state M = S - U so each step is 2 fused
  scalar_tensor_tensor ops per channel-half on DVE:
      U = (M * -beta) + z     (in-place over z in yreg)
      M = (U > 1) - U
  (bit-identical rounding to the reference serial scan). Spikes are extracted
  per 128-step chunk by the GpSimd engine (is_gt -> fp16 x / xs for the next
  layer), which lets the NEXT layer's conv matmuls stream on the PE while the
  scan is still running.
"""

import os
import numpy as np

T, B, J, H, K, NL = 512, 64, 256, 256, 25, 3
THETA = 1.0
SIGMA_INIT = 0.5
EPS = 1e-5
BL = B // 8          # batch per core
NBLK = 2 * BL        # (b, c) blocks per core
TPAD = T + (K - 1)   # left-padded time for conv input
TC = 128             # streaming chunk (layers 1,2)
NQ = T // TC

_CACHE = {}
LAST = {"exec_time_ns": None, "results": None}


# ----------------------------------------------------------------------------
# Host-side math
# ----------------------------------------------------------------------------

def _gauss_kernel_host(W, P):
    """Replicates reference.gauss_kernel in fp32. Uses jax-cpu when available
    so host arithmetic bit-matches the jax reference; numpy fallback."""
    try:
        import jax
        import jax.numpy as jnp

        cpu = jax.devices("cpu")[0]

        def gk(W, P):
            pos = jnp.arange(K, dtype=W.dtype)
            c = P + K // 2
            s = jnp.abs(jnp.float32(SIGMA_INIT)) + 0.27
            g = jnp.exp(-0.5 * ((pos[None, None, :] - c[..., None]) / s) ** 2)
            g = g / (jnp.sum(g, axis=-1, keepdims=True) + 1e-7)
            return W[..., None] * g

        with jax.default_device(cpu):
            return np.array(jax.jit(gk, backend="cpu")(jnp.asarray(W), jnp.asarray(P)))
    except Exception:
        pos = np.arange(K, dtype=np.float32)
        c = (P + np.float32(K // 2)).astype(np.float32)
        s = np.float32(abs(SIGMA_INIT) + 0.27)
        t = ((pos[None, None, :] - c[..., None]) / s).astype(np.float32)
        g = np.exp((np.float32(-0.5) * (t * t)).astype(np.float32)).astype(np.float32)
        den = (np.sum(g, axis=-1, keepdims=True, dtype=np.float32) + np.float32(1e-7)).astype(np.float32)
        g = (g / den).astype(np.float32)
        return (W[..., None] * g).astype(np.float32)


def _fp16_split(kern):
    """kern (fp32) -> (w0, w1) fp16 with kern ~= w0 + w1 * 2^-12 (residual
    <= ~2^-24*|kern|). Subnormal fp16 values are flushed host-side so PE
    flush-to-zero behavior (if any) cannot bite."""
    FP16_MIN_NORMAL = 6.104e-5
    w0 = kern.astype(np.float16)
    w0 = np.where(np.abs(w0.astype(np.float32)) < FP16_MIN_NORMAL, np.float16(0), w0)
    r = (kern - w0.astype(np.float32)) * np.float32(4096.0)
    w1 = r.astype(np.float16)
    w1 = np.where(np.abs(w1.astype(np.float32)) < FP16_MIN_NORMAL, np.float16(0), w1)
    return w0, w1


def _prep_static(W, P, beta, gamma, bb):
    """Build the replicated parameter arrays (same on all cores)."""
    # weights: (l, c_outhalf, split, j_partition, jc, k, h) fp16
    wts = np.empty((NL, 2, 2, 128, 2, K, 128), np.float16)
    for l in range(NL):
        kern = _gauss_kernel_host(W[l], P[l])      # (H, J, K) fp32
        w0, w1 = _fp16_split(kern)
        for c in range(2):
            for s, wsrc in enumerate((w0, w1)):
                blk = wsrc[c * 128:(c + 1) * 128]          # (128h, J, K)
                arr = blk.reshape(128, 2, 128, K)          # (h, jc, j, k)
                wts[l, c, s] = arr.transpose(2, 1, 3, 0)   # (j, jc, k, h)

    negb = np.empty((NL, 128, 2), np.float32)
    omb = np.empty((NL, 128, 2), np.float32)
    gcol = np.empty((NL, 128, 2), np.float32)
    bcol = np.empty((NL, 128, 2), np.float32)
    for l in range(NL):
        for c in range(2):
            ch = beta[l, c * 128:(c + 1) * 128].astype(np.float32)
            negb[l, :, c] = -ch
            omb[l, :, c] = (np.float32(1.0) - ch).astype(np.float32)
            gcol[l, :, c] = gamma[l, c * 128:(c + 1) * 128]
            bcol[l, :, c] = bb[l, c * 128:(c + 1) * 128]
    return wts, negb, omb, gcol, bcol


def _prep_percore(x, U0, core):
    x16 = np.zeros((128, NBLK, TPAD), np.float16)
    xs = x[:, core * BL:(core + 1) * BL, :]            # (T, BL, J)
    a = xs.astype(np.float16).reshape(T, BL, 2, 128)   # (t, b, jc, p)
    x16[:, :, K - 1:] = a.transpose(3, 1, 2, 0).reshape(128, NBLK, T)
    xs16 = (x16.astype(np.float32) * np.float32(2.0 ** -12)).astype(np.float16)

    negu0 = np.empty((NL, 128, 16), np.float32)
    u = U0[:, core * BL:(core + 1) * BL, :]            # (NL, BL, H)
    for l in range(NL):
        a = u[l].reshape(BL, 2, 128)                   # (b, c, p)
        negu0[l] = -a.transpose(2, 0, 1).reshape(128, 16)
    return x16, xs16, negu0


# ----------------------------------------------------------------------------
# Device program
# ----------------------------------------------------------------------------

def _build(nc, bn_affine_trivial):
    import concourse.tile as tile
    from concourse import mybir
    from contextlib import ExitStack

    F32 = mybir.dt.float32
    F16 = mybir.dt.float16
    AX = mybir.AxisListType
    OP = mybir.AluOpType
    ACT = mybir.ActivationFunctionType

    ap_x = nc.dram_tensor("x16", [128, NBLK, TPAD], F16, kind="ExternalInput").ap()
    ap_xs = nc.dram_tensor("xs16", [128, NBLK, TPAD], F16, kind="ExternalInput").ap()
    ap_w = nc.dram_tensor("wts", [NL, 2, 2, 128, 2, K, 128], F16, kind="ExternalInput").ap()
    ap_nu0 = nc.dram_tensor("negu0", [NL, 128, 16], F32, kind="ExternalInput").ap()
    ap_nb = nc.dram_tensor("negb", [NL, 128, 2], F32, kind="ExternalInput").ap()
    ap_omb = nc.dram_tensor("omb", [NL, 128, 2], F32, kind="ExternalInput").ap()
    ap_g = nc.dram_tensor("gcol", [NL, 128, 2], F32, kind="ExternalInput").ap()
    ap_bb = nc.dram_tensor("bcol", [NL, 128, 2], F32, kind="ExternalInput").ap()
    ap_out = nc.dram_tensor("out", [2, 128, BL, T], F32, kind="ExternalOutput").ap()

    with tile.TileContext(nc) as tc, ExitStack() as ctx:
        wp = ctx.enter_context(tc.tile_pool(name="wp", bufs=2))
        xp = ctx.enter_context(tc.tile_pool(name="xp", bufs=1))
        yp = ctx.enter_context(tc.tile_pool(name="yp", bufs=1))
        sp = ctx.enter_context(tc.tile_pool(name="sp", bufs=2))
        st = ctx.enter_context(tc.tile_pool(name="st", bufs=2))
        ps = ctx.enter_context(tc.tile_pool(name="ps", bufs=1, space="PSUM"))
        dr = ctx.enter_context(tc.tile_pool(name="dr", bufs=4, space="DRAM"))

        xt = xp.tile([128, NBLK, TPAD], F16, tag="x")
        xst = xp.tile([128, NBLK, TPAD], F16, tag="xs")
        yreg = yp.tile([128, NBLK, T], F32, tag="yreg")
        nc.sync.dma_start(xt[:], ap_x)
        nc.sync.dma_start(xst[:], ap_xs)

        # warmup collective: absorbs inter-core start skew off the critical path
        win = dr.tile([128, 4], F32, tag="warm")
        wout = dr.tile([128, 4], F32, tag="warm")
        wsrc = st.tile([128, 4], F32, tag="wsrc")
        nc.vector.memset(wsrc[:], 0.0)
        nc.sync.dma_start(win[:], wsrc[:])
        nc.gpsimd.collective_compute(
            "AllReduce", OP.add, replica_groups=[list(range(8))],
            ins=[win[:].opt()], outs=[wout[:].opt()])

        # layer-0 weights now; later layers prefetched during the prior conv
        wtiles = {}
        for c in range(2):
            for s in range(2):
                wtile = wp.tile([128, 2, K, 128], F16, tag=f"w{c}{s}")
                nc.sync.dma_start(wtile[:], ap_w[0, c, s])
                wtiles[(c, s)] = wtile

        sqscr = sp.tile([128, 512], F32, tag="sqscr")

        for l in range(NL):
            chunks = [(0, T)] if l == 0 else [(q * TC, TC) for q in range(NQ)]
            ncols = 16 * len(chunks)
            sacc = st.tile([128, 64], F32, tag="sacc")
            qacc = st.tile([128, 64], F32, tag="qacc")

            # ---------------- conv (streamed in chunks for l>0) ----------------
            for qi, (t0, tw) in enumerate(chunks):
                for c in range(2):
                    pts = []
                    for b in range(BL):
                        pt = ps.tile([128, 512], F32, tag=f"p{b}")
                        pts.append(pt)
                    for s in range(2):
                        xsrc = xt if s == 0 else xst
                        w_cs = wtiles[(c, s)]
                        for jc in range(2):
                            for k in range(K):
                                first = (s == 0 and jc == 0 and k == 0)
                                last = (s == 1 and jc == 1 and k == K - 1)
                                for b in range(BL):
                                    nc.tensor.matmul(
                                        pts[b][:, 0:tw],
                                        lhsT=w_cs[:, jc, k, :],
                                        rhs=xsrc[:, b * 2 + jc, t0 + k:t0 + k + tw],
                                        start=first, stop=last)
                    for b in range(BL):
                        col = qi * 16 + b * 2 + c
                        nc.scalar.activation(
                            yreg[:, b * 2 + c, t0:t0 + tw], pts[b][:, 0:tw],
                            ACT.Copy, accum_out=sacc[:, col:col + 1])
                        nc.scalar.activation(
                            sqscr[:, 0:tw], pts[b][:, 0:tw],
                            ACT.Square, accum_out=qacc[:, col:col + 1])

            # prefetch next layer's weights (DMA overlaps BN + scan)
            if l + 1 < NL:
                for c in range(2):
                    for s in range(2):
                        wtile = wp.tile([128, 2, K, 128], F16, tag=f"w{c}{s}")
                        nc.sync.dma_start(wtile[:], ap_w[l + 1, c, s])
                        wtiles[(c, s)] = wtile

            # ---------------- BN stats: single AllReduce of [sum, sumsq] -------
            ssum = st.tile([128, 4], F32, tag="ssum")
            for c in range(2):
                nc.vector.tensor_reduce(ssum[:, c:c + 1], sacc[:, c:ncols:2], AX.X, OP.add)
                nc.vector.tensor_reduce(ssum[:, 2 + c:3 + c], qacc[:, c:ncols:2], AX.X, OP.add)
            cin = dr.tile([128, 4], F32, tag="cc")
            cout = dr.tile([128, 4], F32, tag="cc")
            nc.sync.dma_start(cin[:], ssum[:])
            nc.gpsimd.collective_compute(
                "AllReduce", OP.add, replica_groups=[list(range(8))],
                ins=[cin[:].opt()], outs=[cout[:].opt()])
            gsum = st.tile([128, 4], F32, tag="gsum")
            nc.sync.dma_start(gsum[:], cout[:])

            mu = st.tile([128, 2], F32, tag="mu")
            v = st.tile([128, 2], F32, tag="v")
            nc.vector.tensor_scalar(mu[:], gsum[:, 0:2], float(1.0 / (T * B)), None, OP.mult)
            nc.vector.tensor_scalar(v[:], gsum[:, 2:4], float(1.0 / (T * B)), None, OP.mult)
            musq = st.tile([128, 2], F32, tag="musq")
            nc.vector.tensor_tensor(musq[:], mu[:], mu[:], OP.mult)
            nc.vector.tensor_tensor(v[:], v[:], musq[:], OP.subtract)
            nc.vector.tensor_scalar(v[:], v[:], float(EPS), None, OP.add)

            # s = sqrt(v) via ACT seed + 2 Newton iters; r = 1/s via DVE divide
            sqt = st.tile([128, 2], F32, tag="sqt")
            rcp = st.tile([128, 2], F32, tag="rcp")
            qt = st.tile([128, 2], F32, tag="qt")
            nc.scalar.activation(sqt[:], v[:], ACT.Sqrt)
            for _ in range(2):
                nc.vector.reciprocal(rcp[:], sqt[:])
                nc.vector.tensor_tensor(qt[:], v[:], rcp[:], OP.mult)
                nc.vector.tensor_tensor(sqt[:], sqt[:], qt[:], OP.add)
                nc.vector.tensor_scalar(sqt[:], sqt[:], 0.5, None, OP.mult)
            rr = st.tile([128, 2], F32, tag="rr")
            nc.vector.reciprocal(rr[:], sqt[:])

            # ---------------- z = ((y-mu)*r [*gamma +bb]) * (1-beta) ----------
            ombc = st.tile([128, 2], F32, tag="ombc")
            nc.sync.dma_start(ombc[:], ap_omb[l])
            if not bn_affine_trivial:
                gc = st.tile([128, 2], F32, tag="gc")
                bc = st.tile([128, 2], F32, tag="bc")
                nc.sync.dma_start(gc[:], ap_g[l])
                nc.sync.dma_start(bc[:], ap_bb[l])
            for c in range(2):
                blk = yreg[:, c::2, :]
                nc.vector.tensor_scalar(blk, blk, mu[:, c:c + 1], rr[:, c:c + 1],
                                        OP.subtract, OP.mult)
                if not bn_affine_trivial:
                    nc.vector.tensor_scalar(blk, blk, gc[:, c:c + 1], bc[:, c:c + 1],
                                            OP.mult, OP.add)
                nc.vector.tensor_scalar(blk, blk, ombc[:, c:c + 1], None, OP.mult)

            # ---------------- LIF scan: M = S - U carried, 2 STT/step/half ----
            nbt = st.tile([128, 2], F32, tag="nbt")
            nu0 = st.tile([128, 16], F32, tag="nu0")
            M0t = st.tile([128, 8], F32, tag="M0t")
            M1t = st.tile([128, 8], F32, tag="M1t")
            nc.sync.dma_start(nbt[:], ap_nb[l])
            nc.sync.dma_start(nu0[:], ap_nu0[l])
            nc.vector.tensor_copy(M0t[:], nu0[:, 0::2])
            nc.vector.tensor_copy(M1t[:], nu0[:, 1::2])
            Ms = (M0t, M1t)
            for t in range(T):
                z0 = yreg[:, 0::2, t]
                z1 = yreg[:, 1::2, t]
                nc.vector.scalar_tensor_tensor(z0, Ms[0][:], nbt[:, 0:1], z0,
                                               OP.mult, OP.add)
                nc.vector.scalar_tensor_tensor(z1, Ms[1][:], nbt[:, 1:2], z1,
                                               OP.mult, OP.add)
                nc.vector.scalar_tensor_tensor(Ms[0][:], z0, float(THETA), z0,
                                               OP.is_gt, OP.subtract)
                nc.vector.scalar_tensor_tensor(Ms[1][:], z1, float(THETA), z1,
                                               OP.is_gt, OP.subtract)
                if (t + 1) % TC == 0:
                    q0 = (t + 1) - TC
                    uch = yreg[:, :, q0:t + 1]
                    if l < NL - 1:
                        # spikes -> next layer's x (fp16) and xs (fp16 * 2^-12)
                        for c in range(2):
                            nc.gpsimd.tensor_scalar(
                                xt[:, c::2, K - 1 + q0:K - 1 + t + 1],
                                uch[:, c::2, :], float(THETA), None, OP.is_gt)
                            nc.gpsimd.tensor_scalar(
                                xst[:, c::2, K - 1 + q0:K - 1 + t + 1],
                                uch[:, c::2, :], float(THETA), float(2.0 ** -12),
                                OP.is_gt, OP.mult)
                    else:
                        # final layer: spikes fp32 in place, stream out
                        nc.gpsimd.tensor_scalar(uch, uch, float(THETA), None, OP.is_gt)
                        for c in range(2):
                            nc.sync.dma_start(ap_out[c][:, :, q0:t + 1],
                                              yreg[:, c::2, q0:t + 1])
    nc.compile()
    return nc


def _get_compiled(bn_affine_trivial):
    key = ("prog", bn_affine_trivial)
    if key not in _CACHE:
        from concourse import bacc
        nc = bacc.Bacc("TRN2", target_bir_lowering=False, debug=False, num_devices=8)
        _CACHE[key] = _build(nc, bn_affine_trivial)
    return _CACHE[key]


# ----------------------------------------------------------------------------
# Profiled run (dev-only; needs the axon NTFF side channel)
# ----------------------------------------------------------------------------

def _run_profiled(nc, in_maps):
    import glob
    import tempfile
    from concourse.bass_utils import run_bass_kernel_spmd

    prof = {}
    try:
        from trn_agent_boot.trn_boot import _ntff_profile_via_ctypes
        hook = _ntff_profile_via_ctypes("/opt/axon/libaxon_pjrt.so")
        assert hook is not None
        neff_dir = tempfile.mkdtemp(prefix="snn_ntff_")
        with hook(neff_dir, [0]):
            res = run_bass_kernel_spmd(nc, in_maps, list(range(8)))
        ntffs = glob.glob(os.path.join(neff_dir, "*_body*.ntff"))
        prof["neff_dir"] = neff_dir
        if ntffs:
            import gauge.profiler
            from concourse._compat import FishPath
            p = gauge.profiler.Profile(
                profile_path=FishPath(neff_dir), kernel_dev_mode=True,
                profile_on_exit=False, bass_kernel=nc.m,
                offline_processing=True, fname="*_body*")
            rs = p.to_perfetto(model_index=(0,))
            if rs:
                prof["exec_time_ns"] = rs[0].exec_time_ns
                prof["trace_path"] = str(rs[0].trace_path)
                prof["scope_times"] = dict(rs[0].scope_times)
        return res, prof
    except Exception as e:  # profiling is best-effort
        prof["error"] = repr(e)
        res = run_bass_kernel_spmd(nc, in_maps, list(range(8)))
        return res, prof


# ----------------------------------------------------------------------------
# Entry point
# ----------------------------------------------------------------------------

def kernel(x, W, P, beta, gamma, bb, U0):
    from concourse.bass_utils import run_bass_kernel_spmd

    x = np.asarray(x, np.float32)
    W = np.asarray(W, np.float32)
    P = np.asarray(P, np.float32)
    beta = np.asarray(beta, np.float32)
    gamma = np.asarray(gamma, np.float32)
    bb = np.asarray(bb, np.float32)
    U0 = np.asarray(U0, np.float32)

    trivial = bool(np.all(gamma == 1.0) and np.all(bb == 0.0))
    nc = _get_compiled(trivial)

    skey = ("static", W.tobytes(), P.tobytes(), beta.tobytes(),
            gamma.tobytes(), bb.tobytes())
    sk = hash(skey)
    if _CACHE.get("static_key") != sk:
        _CACHE["static"] = _prep_static(W, P, beta, gamma, bb)
        _CACHE["static_key"] = sk
    wts, negb, omb, gcol, bcol = _CACHE["static"]

    in_maps = []
    for core in range(8):
        x16, xs16, negu0 = _prep_percore(x, U0, core)
        in_maps.append(dict(x16=x16, xs16=xs16, wts=wts, negu0=negu0,
                            negb=negb, omb=omb, gcol=gcol, bcol=bcol))

    trace = bool(int(os.environ.get("BASS_SNN_TRACE", "0")))
    if trace:
        res, prof = _run_profiled(nc, in_maps)
        LAST["exec_time_ns"] = prof.get("exec_time_ns")
        LAST["profile"] = prof
    else:
        res = run_bass_kernel_spmd(nc, in_maps, list(range(8)))
        LAST["exec_time_ns"] = res.exec_time_ns
    LAST["results"] = res

    o = np.empty((T, B, H), np.float32)
    for core in range(8):
        arr = res.results[core]["out"]                  # (2, 128, BL, T)
        o[:, core * BL:(core + 1) * BL, :] = (
            arr.transpose(3, 2, 0, 1).reshape(T, BL, H))
    return o


# revision 4
# speedup vs baseline: 1.9524x; 1.2217x over previous
"""Trainium2 Bass kernel for nn_DelayLMLIFSNN (3-layer delay-conv + BN + LIF SNN).

Strategy (v2):
- Data-parallel over batch B=64 across 8 NeuronCores (8 batch elements/core).
- Conv: causal dilated-gauss conv as PE matmuls, fp16 weight-split
  (w = w0 + w1*2^-12) accumulated in a SINGLE psum chain per output block by
  pre-scaling a second copy of the (binary, fp16-exact) input: xs = x*2^-12.
- BN stats fused into the conv epilogue on the Scalar engine (Copy+accum for
  the sum, Square+accum for the sumsq), one-pass var = E[y^2]-mu^2, a single
  [128,4] AllReduce per layer, plus a warmup AllReduce at kernel start that
  absorbs inter-core start skew.
- LIF scan: per-step state M = S - U so each step is 2 fused
  scalar_tensor_tensor ops per channel-half on DVE:
      U = (M * -beta) + z     (in-place over z in yreg)
      M = (U > 1) - U
  (bit-identical rounding to the reference serial scan). Spikes are extracted
  per 128-step chunk by the GpSimd engine (is_gt -> fp16 x / xs for the next
  layer), which lets the NEXT layer's conv matmuls stream on the PE while the
  scan is still running.
"""

import os
import numpy as np

T, B, J, H, K, NL = 512, 64, 256, 256, 25, 3
THETA = 1.0
SIGMA_INIT = 0.5
EPS = 1e-5
BL = B // 8          # batch per core
NBLK = 2 * BL        # (b, c) blocks per core
TPAD = T + (K - 1)   # left-padded time for conv input
TC = 128             # streaming chunk (layers 1,2)
NQ = T // TC

_CACHE = {}
LAST = {"exec_time_ns": None, "results": None}


# ----------------------------------------------------------------------------
# Host-side math
# ----------------------------------------------------------------------------

def _gauss_kernel_host(W, P):
    """Replicates reference.gauss_kernel in fp32. Uses jax-cpu when available
    so host arithmetic bit-matches the jax reference; numpy fallback."""
    try:
        import jax
        import jax.numpy as jnp

        cpu = jax.devices("cpu")[0]

        def gk(W, P):
            pos = jnp.arange(K, dtype=W.dtype)
            c = P + K // 2
            s = jnp.abs(jnp.float32(SIGMA_INIT)) + 0.27
            g = jnp.exp(-0.5 * ((pos[None, None, :] - c[..., None]) / s) ** 2)
            g = g / (jnp.sum(g, axis=-1, keepdims=True) + 1e-7)
            return W[..., None] * g

        with jax.default_device(cpu):
            return np.array(jax.jit(gk, backend="cpu")(jnp.asarray(W), jnp.asarray(P)))
    except Exception:
        pos = np.arange(K, dtype=np.float32)
        c = (P + np.float32(K // 2)).astype(np.float32)
        s = np.float32(abs(SIGMA_INIT) + 0.27)
        t = ((pos[None, None, :] - c[..., None]) / s).astype(np.float32)
        g = np.exp((np.float32(-0.5) * (t * t)).astype(np.float32)).astype(np.float32)
        den = (np.sum(g, axis=-1, keepdims=True, dtype=np.float32) + np.float32(1e-7)).astype(np.float32)
        g = (g / den).astype(np.float32)
        return (W[..., None] * g).astype(np.float32)


def _fp16_split(kern):
    """kern (fp32) -> (w0, w1) fp16 with kern ~= w0 + w1 * 2^-12 (residual
    <= ~2^-24*|kern|). Subnormal fp16 values are flushed host-side so PE
    flush-to-zero behavior (if any) cannot bite."""
    FP16_MIN_NORMAL = 6.104e-5
    w0 = kern.astype(np.float16)
    w0 = np.where(np.abs(w0.astype(np.float32)) < FP16_MIN_NORMAL, np.float16(0), w0)
    r = (kern - w0.astype(np.float32)) * np.float32(4096.0)
    w1 = r.astype(np.float16)
    w1 = np.where(np.abs(w1.astype(np.float32)) < FP16_MIN_NORMAL, np.float16(0), w1)
    return w0, w1


def _prep_static(W, P, beta, gamma, bb):
    """Build the replicated parameter arrays (same on all cores)."""
    # weights: (l, c_outhalf, split, j_partition, jc, k, h) fp16
    wts = np.empty((NL, 2, 2, 128, 2, K, 128), np.float16)
    for l in range(NL):
        kern = _gauss_kernel_host(W[l], P[l])      # (H, J, K) fp32
        w0, w1 = _fp16_split(kern)
        for c in range(2):
            for s, wsrc in enumerate((w0, w1)):
                blk = wsrc[c * 128:(c + 1) * 128]          # (128h, J, K)
                arr = blk.reshape(128, 2, 128, K)          # (h, jc, j, k)
                wts[l, c, s] = arr.transpose(2, 1, 3, 0)   # (j, jc, k, h)

    negb = np.empty((NL, 128, 2), np.float32)
    omb = np.empty((NL, 128, 2), np.float32)
    gcol = np.empty((NL, 128, 2), np.float32)
    bcol = np.empty((NL, 128, 2), np.float32)
    for l in range(NL):
        for c in range(2):
            ch = beta[l, c * 128:(c + 1) * 128].astype(np.float32)
            negb[l, :, c] = -ch
            omb[l, :, c] = (np.float32(1.0) - ch).astype(np.float32)
            gcol[l, :, c] = gamma[l, c * 128:(c + 1) * 128]
            bcol[l, :, c] = bb[l, c * 128:(c + 1) * 128]
    return wts, negb, omb, gcol, bcol


def _prep_percore(x, U0, core):
    x16 = np.zeros((128, NBLK, TPAD), np.float16)
    xs = x[:, core * BL:(core + 1) * BL, :]            # (T, BL, J)
    a = xs.astype(np.float16).reshape(T, BL, 2, 128)   # (t, b, jc, p)
    x16[:, :, K - 1:] = a.transpose(3, 1, 2, 0).reshape(128, NBLK, T)
    xs16 = (x16.astype(np.float32) * np.float32(2.0 ** -12)).astype(np.float16)

    negu0 = np.empty((NL, 128, 16), np.float32)
    u = U0[:, core * BL:(core + 1) * BL, :]            # (NL, BL, H)
    for l in range(NL):
        a = u[l].reshape(BL, 2, 128)                   # (b, c, p)
        negu0[l] = -a.transpose(2, 0, 1).reshape(128, 16)
    return x16, xs16, negu0


# ----------------------------------------------------------------------------
# Device program
# ----------------------------------------------------------------------------

def _build(nc, bn_affine_trivial):
    import concourse.tile as tile
    from concourse import mybir
    from contextlib import ExitStack

    F32 = mybir.dt.float32
    F16 = mybir.dt.float16
    AX = mybir.AxisListType
    OP = mybir.AluOpType
    ACT = mybir.ActivationFunctionType

    ap_x = nc.dram_tensor("x16", [128, NBLK, TPAD], F16, kind="ExternalInput").ap()
    ap_xs = nc.dram_tensor("xs16", [128, NBLK, TPAD], F16, kind="ExternalInput").ap()
    ap_w = nc.dram_tensor("wts", [NL, 2, 2, 128, 2, K, 128], F16, kind="ExternalInput").ap()
    ap_nu0 = nc.dram_tensor("negu0", [NL, 128, 16], F32, kind="ExternalInput").ap()
    ap_nb = nc.dram_tensor("negb", [NL, 128, 2], F32, kind="ExternalInput").ap()
    ap_omb = nc.dram_tensor("omb", [NL, 128, 2], F32, kind="ExternalInput").ap()
    ap_g = nc.dram_tensor("gcol", [NL, 128, 2], F32, kind="ExternalInput").ap()
    ap_bb = nc.dram_tensor("bcol", [NL, 128, 2], F32, kind="ExternalInput").ap()
    ap_out = nc.dram_tensor("out", [2, 128, BL, T], F32, kind="ExternalOutput").ap()

    with tile.TileContext(nc) as tc, ExitStack() as ctx:
        wp = ctx.enter_context(tc.tile_pool(name="wp", bufs=2))
        xp = ctx.enter_context(tc.tile_pool(name="xp", bufs=1))
        yp = ctx.enter_context(tc.tile_pool(name="yp", bufs=1))
        sp = ctx.enter_context(tc.tile_pool(name="sp", bufs=2))
        st = ctx.enter_context(tc.tile_pool(name="st", bufs=2))
        ps = ctx.enter_context(tc.tile_pool(name="ps", bufs=1, space="PSUM"))
        dr = ctx.enter_context(tc.tile_pool(name="dr", bufs=4, space="DRAM"))

        xt = xp.tile([128, NBLK, TPAD], F16, tag="x")
        xst = xp.tile([128, NBLK, TPAD], F16, tag="xs")
        yreg = yp.tile([128, NBLK, T], F32, tag="yreg")
        nc.sync.dma_start(xt[:], ap_x)
        nc.sync.dma_start(xst[:], ap_xs)

        # warmup collective: absorbs inter-core start skew off the critical path
        win = dr.tile([128, 4], F32, tag="warm")
        wout = dr.tile([128, 4], F32, tag="warm")
        wsrc = st.tile([128, 4], F32, tag="wsrc")
        nc.vector.memset(wsrc[:], 0.0)
        nc.sync.dma_start(win[:], wsrc[:])
        nc.gpsimd.collective_compute(
            "AllReduce", OP.add, replica_groups=[list(range(8))],
            ins=[win[:].opt()], outs=[wout[:].opt()])

        # layer-0 weights now; later layers prefetched during the prior conv
        wtiles = {}
        for c in range(2):
            for s in range(2):
                wtile = wp.tile([128, 2, K, 128], F16, tag=f"w{c}{s}")
                nc.sync.dma_start(wtile[:], ap_w[0, c, s])
                wtiles[(c, s)] = wtile

        sqscr = sp.tile([128, 512], F32, tag="sqscr")

        for l in range(NL):
            chunks = [(0, T)] if l == 0 else [(q * TC, TC) for q in range(NQ)]
            ncols = 16 * len(chunks)
            sacc = st.tile([128, 64], F32, tag="sacc")
            qacc = st.tile([128, 64], F32, tag="qacc")

            # ---------------- conv (streamed in chunks for l>0) ----------------
            for qi, (t0, tw) in enumerate(chunks):
                for c in range(2):
                    pts = []
                    for b in range(BL):
                        pt = ps.tile([128, 512], F32, tag=f"p{b}")
                        pts.append(pt)
                    for s in range(2):
                        xsrc = xt if s == 0 else xst
                        w_cs = wtiles[(c, s)]
                        for jc in range(2):
                            for k in range(K):
                                first = (s == 0 and jc == 0 and k == 0)
                                last = (s == 1 and jc == 1 and k == K - 1)
                                for b in range(BL):
                                    nc.tensor.matmul(
                                        pts[b][:, 0:tw],
                                        lhsT=w_cs[:, jc, k, :],
                                        rhs=xsrc[:, b * 2 + jc, t0 + k:t0 + k + tw],
                                        start=first, stop=last)
                    for b in range(BL):
                        col = qi * 16 + b * 2 + c
                        nc.scalar.activation(
                            yreg[:, b * 2 + c, t0:t0 + tw], pts[b][:, 0:tw],
                            ACT.Copy, accum_out=sacc[:, col:col + 1])
                        nc.scalar.activation(
                            sqscr[:, 0:tw], pts[b][:, 0:tw],
                            ACT.Square, accum_out=qacc[:, col:col + 1])

            # prefetch next layer's weights (DMA overlaps BN + scan)
            if l + 1 < NL:
                for c in range(2):
                    for s in range(2):
                        wtile = wp.tile([128, 2, K, 128], F16, tag=f"w{c}{s}")
                        nc.sync.dma_start(wtile[:], ap_w[l + 1, c, s])
                        wtiles[(c, s)] = wtile

            # ---------------- BN stats: single AllReduce of [sum, sumsq] -------
            ssum = st.tile([128, 4], F32, tag="ssum")
            for c in range(2):
                nc.vector.tensor_reduce(ssum[:, c:c + 1], sacc[:, c:ncols:2], AX.X, OP.add)
                nc.vector.tensor_reduce(ssum[:, 2 + c:3 + c], qacc[:, c:ncols:2], AX.X, OP.add)
            cin = dr.tile([128, 4], F32, tag="cc")
            cout = dr.tile([128, 4], F32, tag="cc")
            nc.sync.dma_start(cin[:], ssum[:])
            nc.gpsimd.collective_compute(
                "AllReduce", OP.add, replica_groups=[list(range(8))],
                ins=[cin[:].opt()], outs=[cout[:].opt()])
            gsum = st.tile([128, 4], F32, tag="gsum")
            nc.sync.dma_start(gsum[:], cout[:])

            mu = st.tile([128, 2], F32, tag="mu")
            v = st.tile([128, 2], F32, tag="v")
            nc.vector.tensor_scalar(mu[:], gsum[:, 0:2], float(1.0 / (T * B)), None, OP.mult)
            nc.vector.tensor_scalar(v[:], gsum[:, 2:4], float(1.0 / (T * B)), None, OP.mult)
            musq = st.tile([128, 2], F32, tag="musq")
            nc.vector.tensor_tensor(musq[:], mu[:], mu[:], OP.mult)
            nc.vector.tensor_tensor(v[:], v[:], musq[:], OP.subtract)
            nc.vector.tensor_scalar(v[:], v[:], float(EPS), None, OP.add)

            # s = sqrt(v) via ACT seed + 2 Newton iters; r = 1/s via DVE divide
            sqt = st.tile([128, 2], F32, tag="sqt")
            rcp = st.tile([128, 2], F32, tag="rcp")
            qt = st.tile([128, 2], F32, tag="qt")
            nc.scalar.activation(sqt[:], v[:], ACT.Sqrt)
            for _ in range(2):
                nc.vector.reciprocal(rcp[:], sqt[:])
                nc.vector.tensor_tensor(qt[:], v[:], rcp[:], OP.mult)
                nc.vector.tensor_tensor(sqt[:], sqt[:], qt[:], OP.add)
                nc.vector.tensor_scalar(sqt[:], sqt[:], 0.5, None, OP.mult)
            rr = st.tile([128, 2], F32, tag="rr")
            nc.vector.reciprocal(rr[:], sqt[:])

            # ---------------- z = ((y-mu)*r [*gamma +bb]) * (1-beta) ----------
            ombc = st.tile([128, 2], F32, tag="ombc")
            nc.sync.dma_start(ombc[:], ap_omb[l])
            if not bn_affine_trivial:
                gc = st.tile([128, 2], F32, tag="gc")
                bc = st.tile([128, 2], F32, tag="bc")
                nc.sync.dma_start(gc[:], ap_g[l])
                nc.sync.dma_start(bc[:], ap_bb[l])
                for c in range(2):
                    blk = yreg[:, c::2, :]
                    nc.vector.tensor_scalar(blk, blk, mu[:, c:c + 1], rr[:, c:c + 1],
                                            OP.subtract, OP.mult)
                    nc.vector.tensor_scalar(blk, blk, gc[:, c:c + 1], bc[:, c:c + 1],
                                            OP.mult, OP.add)
                    nc.vector.tensor_scalar(blk, blk, ombc[:, c:c + 1], None, OP.mult)
            else:
                # z = y*a + cb on the Scalar engine: a = r*(1-beta), cb = -mu*a
                acol = st.tile([128, 2], F32, tag="acol")
                ccol = st.tile([128, 2], F32, tag="ccol")
                nc.vector.tensor_tensor(acol[:], rr[:], ombc[:], OP.mult)
                nc.vector.tensor_tensor(ccol[:], mu[:], acol[:], OP.mult)
                nc.vector.tensor_scalar(ccol[:], ccol[:], -1.0, None, OP.mult)
                for c in range(2):
                    blk = yreg[:, c::2, :]
                    nc.scalar.activation(blk, blk, ACT.Identity,
                                         bias=ccol[:, c:c + 1],
                                         scale=acol[:, c:c + 1])

            # ---------------- LIF scan: M = S - U carried, 2 STT/step/half ----
            nbt = st.tile([128, 2], F32, tag="nbt")
            nu0 = st.tile([128, 16], F32, tag="nu0")
            M0t = st.tile([128, 8], F32, tag="M0t")
            M1t = st.tile([128, 8], F32, tag="M1t")
            nc.sync.dma_start(nbt[:], ap_nb[l])
            nc.sync.dma_start(nu0[:], ap_nu0[l])
            nc.vector.tensor_copy(M0t[:], nu0[:, 0::2])
            nc.vector.tensor_copy(M1t[:], nu0[:, 1::2])
            Ms = (M0t, M1t)
            for t in range(T):
                z0 = yreg[:, 0::2, t]
                z1 = yreg[:, 1::2, t]
                nc.vector.scalar_tensor_tensor(z0, Ms[0][:], nbt[:, 0:1], z0,
                                               OP.mult, OP.add)
                nc.vector.scalar_tensor_tensor(z1, Ms[1][:], nbt[:, 1:2], z1,
                                               OP.mult, OP.add)
                nc.vector.scalar_tensor_tensor(Ms[0][:], z0, float(THETA), z0,
                                               OP.is_gt, OP.subtract)
                nc.vector.scalar_tensor_tensor(Ms[1][:], z1, float(THETA), z1,
                                               OP.is_gt, OP.subtract)
                if (t + 1) % TC == 0:
                    q0 = (t + 1) - TC
                    uch = yreg[:, :, q0:t + 1]
                    if l < NL - 1:
                        # spikes -> next layer's x (fp16) and xs (fp16 * 2^-12)
                        for c in range(2):
                            nc.vector.tensor_scalar(
                                xt[:, c::2, K - 1 + q0:K - 1 + t + 1],
                                uch[:, c::2, :], float(THETA), None, OP.is_gt)
                        for c in range(2):
                            nc.vector.tensor_scalar(
                                xst[:, c::2, K - 1 + q0:K - 1 + t + 1],
                                uch[:, c::2, :], float(THETA), float(2.0 ** -12),
                                OP.is_gt, OP.mult)
                    else:
                        # final layer: spikes fp32 in place, stream out
                        nc.vector.tensor_scalar(uch, uch, float(THETA), None, OP.is_gt)
                        for c in range(2):
                            nc.sync.dma_start(ap_out[c][:, :, q0:t + 1],
                                              yreg[:, c::2, q0:t + 1])
    nc.compile()
    return nc


def _get_compiled(bn_affine_trivial):
    key = ("prog", bn_affine_trivial)
    if key not in _CACHE:
        from concourse import bacc
        nc = bacc.Bacc("TRN2", target_bir_lowering=False, debug=False, num_devices=8)
        _CACHE[key] = _build(nc, bn_affine_trivial)
    return _CACHE[key]


# ----------------------------------------------------------------------------
# Profiled run (dev-only; needs the axon NTFF side channel)
# ----------------------------------------------------------------------------

def _run_profiled(nc, in_maps):
    import glob
    import tempfile
    from concourse.bass_utils import run_bass_kernel_spmd

    prof = {}
    try:
        from trn_agent_boot.trn_boot import _ntff_profile_via_ctypes
        hook = _ntff_profile_via_ctypes("/opt/axon/libaxon_pjrt.so")
        assert hook is not None
        neff_dir = tempfile.mkdtemp(prefix="snn_ntff_")
        with hook(neff_dir, [0]):
            res = run_bass_kernel_spmd(nc, in_maps, list(range(8)))
        ntffs = glob.glob(os.path.join(neff_dir, "*_body*.ntff"))
        prof["neff_dir"] = neff_dir
        if ntffs:
            import gauge.profiler
            from concourse._compat import FishPath
            p = gauge.profiler.Profile(
                profile_path=FishPath(neff_dir), kernel_dev_mode=True,
                profile_on_exit=False, bass_kernel=nc.m,
                offline_processing=True, fname="*_body*")
            rs = p.to_perfetto(model_index=(0,))
            if rs:
                prof["exec_time_ns"] = rs[0].exec_time_ns
                prof["trace_path"] = str(rs[0].trace_path)
                prof["scope_times"] = dict(rs[0].scope_times)
        return res, prof
    except Exception as e:  # profiling is best-effort
        prof["error"] = repr(e)
        res = run_bass_kernel_spmd(nc, in_maps, list(range(8)))
        return res, prof


# ----------------------------------------------------------------------------
# Entry point
# ----------------------------------------------------------------------------

def kernel(x, W, P, beta, gamma, bb, U0):
    from concourse.bass_utils import run_bass_kernel_spmd

    x = np.asarray(x, np.float32)
    W = np.asarray(W, np.float32)
    P = np.asarray(P, np.float32)
    beta = np.asarray(beta, np.float32)
    gamma = np.asarray(gamma, np.float32)
    bb = np.asarray(bb, np.float32)
    U0 = np.asarray(U0, np.float32)

    trivial = bool(np.all(gamma == 1.0) and np.all(bb == 0.0))
    nc = _get_compiled(trivial)

    skey = ("static", W.tobytes(), P.tobytes(), beta.tobytes(),
            gamma.tobytes(), bb.tobytes())
    sk = hash(skey)
    if _CACHE.get("static_key") != sk:
        _CACHE["static"] = _prep_static(W, P, beta, gamma, bb)
        _CACHE["static_key"] = sk
    wts, negb, omb, gcol, bcol = _CACHE["static"]

    in_maps = []
    for core in range(8):
        x16, xs16, negu0 = _prep_percore(x, U0, core)
        in_maps.append(dict(x16=x16, xs16=xs16, wts=wts, negu0=negu0,
                            negb=negb, omb=omb, gcol=gcol, bcol=bcol))

    trace = bool(int(os.environ.get("BASS_SNN_TRACE", "0")))
    if trace:
        res, prof = _run_profiled(nc, in_maps)
        LAST["exec_time_ns"] = prof.get("exec_time_ns")
        LAST["profile"] = prof
    else:
        res = run_bass_kernel_spmd(nc, in_maps, list(range(8)))
        LAST["exec_time_ns"] = res.exec_time_ns
    LAST["results"] = res

    o = np.empty((T, B, H), np.float32)
    for core in range(8):
        arr = res.results[core]["out"]                  # (2, 128, BL, T)
        o[:, core * BL:(core + 1) * BL, :] = (
            arr.transpose(3, 2, 0, 1).reshape(T, BL, H))
    return o
